# Initial kernel scaffold
#
"""Trainium2 Bass kernel for the batch ConsistencyLoss (masked pairwise KL).

Math (reference):
    emb = x / ||x||;  sim = emb @ emb.T;  mask = (sim > 0.8) & ~eye
    L = log_softmax(routing);  P = exp(L);  ne[j] = sum_k P[j,k] L[j,k]
    kl[i,j] = ne[j] - (L @ P.T)[i,j]
    loss = sum(mask * kl) / count(mask)

Factorization used on device (per core, row strip S of 1024 rows):
    sum_{i in S, j} mask[i,j] * kl[i,j]
      = sum_j ne[j] * colcount_S[j] - sum_{j,k} P[j,k] * (mask_S^T @ L_S)[j,k]
    With Lpad = [L_S | 1] (bf16, [i,17]) one PSUM-accumulated matmul
    U = Lpad^T @ mask_tile gives both terms: U[k,j] (k<16) and colcount in U[16,j].
    Diagonal pairs have kl == 0 exactly, so they are left in the mask and only
    the count is corrected by -B on the host.

Per core: big bf16 matmul sim_strip = emb_S @ emb^T ([1024, 8192], K=1024),
thresholded straight out of PSUM into a bf16 0/1 mask, tiny U matmul epilogue.
Embedding normalize+transpose is fused into PE transpose-matmuls against
diag(1/||x||). Host sums the 8 (sum, count) pairs.
"""

import numpy as np

import concourse.bass as bass
import concourse.tile as tile
from concourse import mybir
from concourse.bass_utils import run_bass_kernel_spmd
from concourse.masks import make_identity

B, E, H = 8192, 16, 1024
NCORES = 8
STRIP = B // NCORES  # 1024 rows per core
MT = STRIP // 128    # 8 row chunks per strip
KT = H // 128        # 8 contraction tiles
NT = B // 512        # 16 column tiles of 512
BT = B // 128        # 64 batch tiles
SIM_THRESHOLD = 0.8
WEIGHT = 1.0
F32 = mybir.dt.float32
BF16 = mybir.dt.bfloat16
AX = mybir.AxisListType.X
OP = mybir.AluOpType
AF = mybir.ActivationFunctionType


def _softmax_stats(nc, pool, x, P_out=None, ne_out=None, L_out=None):
    """From logits tile x [128, E]: optionally write P (f32), ne (f32 [128,1])
    and L (any dtype) tiles."""
    negmax = pool.tile([128, 1], F32, tag="negmax")
    nc.vector.reduce_max(out=negmax, in_=x, axis=AX, negate=True)
    e = pool.tile([128, E], F32, tag="e")
    s = pool.tile([128, 1], F32, tag="s")
    nc.scalar.activation(out=e, in_=x, func=AF.Exp, bias=negmax, scale=1.0,
                         accum_out=s)
    logs = pool.tile([128, 1], F32, tag="logs")
    nc.scalar.activation(out=logs, in_=s, func=AF.Ln)
    if L_out is not None:
        # L = (x + negmax) - log(sum)
        nc.vector.tensor_scalar(L_out, x, negmax, logs, op0=OP.add,
                                op1=OP.subtract)
    if P_out is not None:
        rs = pool.tile([128, 1], F32, tag="rs")
        nc.vector.reciprocal(out=rs, in_=s)
        nc.vector.tensor_scalar(P_out, e, rs, None, op0=OP.mult)
    if ne_out is not None:
        L = pool.tile([128, E], F32, tag="Lf")
        nc.vector.tensor_scalar(L, x, negmax, logs, op0=OP.add, op1=OP.subtract)
        scr = pool.tile([128, E], F32, tag="nescr")
        nc.vector.tensor_tensor_reduce(out=scr, in0=P_out, in1=L, scale=1.0,
                                       scalar=0.0, op0=OP.mult, op1=OP.add,
                                       accum_out=ne_out)


def _kernel(tc, emb, emb_s, rp, rp_s, out_dram):
    nc = tc.nc
    with tc.tile_pool(name="persist", bufs=1) as persist:
        embt = persist.tile([128, KT, B], BF16)        # [h%128, h//128, b]
        stript = persist.tile([128, KT, STRIP], BF16)  # strip columns
        P_all = persist.tile([128, BT, E], F32)
        ne_all = persist.tile([128, BT], F32)
        Lpad = persist.tile([128, MT, E + 1], BF16)
        Ut_all = persist.tile([128, BT, E + 1], F32)
        ident = persist.tile([128, 128], F32)
        ones = persist.tile([128, 1], F32)
        make_identity(nc, ident)
        nc.vector.memset(ones, 1.0)
        nc.vector.memset(Lpad[:, :, E], 1.0)

        # ---- Phase A: softmax stats (full batch P/ne; strip Lpad) ----
        with tc.tile_pool(name="smx", bufs=3) as smx:
            for bt in range(BT):
                x = smx.tile([128, E], F32, tag="rpx")
                nc.sync.dma_start(out=x, in_=rp[bt * 128:(bt + 1) * 128, :])
                _softmax_stats(nc, smx, x, P_out=P_all[:, bt, :],
                               ne_out=ne_all[:, bt:bt + 1])
            for ms in range(MT):
                x = smx.tile([128, E], F32, tag="rpx")
                nc.sync.dma_start(out=x, in_=rp_s[ms * 128:(ms + 1) * 128, :])
                _softmax_stats(nc, smx, x, L_out=Lpad[:, ms, 0:E])

        # ---- Phase B: normalize + transpose embeddings ----
        with tc.tile_pool(name="embp", bufs=3) as ep, \
             tc.tile_pool(name="trps", bufs=4, space="PSUM") as trps:

            def prep(src_ap, dst_tile, nb):
                for bt in range(nb):
                    x = ep.tile([128, H], F32, tag="ex")
                    nc.sync.dma_start(out=x,
                                      in_=src_ap[bt * 128:(bt + 1) * 128, :])
                    sq = ep.tile([128, H], F32, tag="sq")
                    ss = ep.tile([128, 1], F32, tag="ss")
                    nc.vector.tensor_tensor_reduce(
                        out=sq, in0=x, in1=x, scale=1.0, scalar=0.0,
                        op0=OP.mult, op1=OP.add, accum_out=ss)
                    norm = ep.tile([128, 1], F32, tag="norm")
                    nc.scalar.sqrt(out=norm, in_=ss)
                    rn = ep.tile([128, 1], F32, tag="rn")
                    nc.vector.reciprocal(out=rn, in_=norm)
                    # diag(1/||x||) for this 128-row chunk
                    ds = ep.tile([128, 128], F32, tag="ds")
                    nc.vector.tensor_scalar(ds, ident, rn, None, op0=OP.mult)
                    for kt in range(KT):
                        tp = trps.tile([128, 128], F32, tag="tr")
                        # out[h, b] = x[b, h] / ||x[b]||
                        nc.tensor.matmul(out=tp,
                                         lhsT=x[:, kt * 128:(kt + 1) * 128],
                                         rhs=ds, start=True, stop=True)
                        eng = nc.vector if kt % 2 == 0 else nc.scalar
                        dst = dst_tile[:, kt, bt * 128:(bt + 1) * 128]
                        if eng is nc.vector:
                            eng.tensor_copy(out=dst, in_=tp)
                        else:
                            eng.copy(out=dst, in_=tp)

            prep(emb, embt, BT)
            prep(emb_s, stript, MT)

        # ---- Phase C: sim strip matmul + mask + U accumulation ----
        with tc.tile_pool(name="simps", bufs=3, space="PSUM") as simps, \
             tc.tile_pool(name="ups", bufs=2, space="PSUM") as ups, \
             tc.tile_pool(name="utps", bufs=2, space="PSUM") as utps, \
             tc.tile_pool(name="mkp", bufs=3) as mkp, \
             tc.tile_pool(name="stg", bufs=2) as stg:
            for n in range(NT):
                u = ups.tile([E + 1, 512], F32, tag="u")
                for m in range(MT):
                    sim = simps.tile([128, 512], F32, tag="sim")
                    for kt in range(KT):
                        nc.tensor.matmul(
                            out=sim,
                            lhsT=stript[:, kt, m * 128:(m + 1) * 128],
                            rhs=embt[:, kt, n * 512:(n + 1) * 512],
                            start=(kt == 0), stop=(kt == KT - 1))
                    msk = mkp.tile([128, 512], BF16, tag="mask")
                    nc.vector.tensor_scalar(msk, sim, SIM_THRESHOLD, None,
                                            op0=OP.is_gt)
                    nc.tensor.matmul(out=u, lhsT=Lpad[:, m, :], rhs=msk,
                                     start=(m == 0), stop=(m == MT - 1))
                # stage U and transpose 128-column blocks into Ut_all
                ust = stg.tile([E + 1, 512], F32, tag="ust")
                nc.scalar.copy(out=ust, in_=u)
                for c in range(4):
                    jt = n * 4 + c
                    tp = utps.tile([128, E + 1], F32, tag="ut")
                    nc.tensor.matmul(out=tp,
                                     lhsT=ust[:, c * 128:(c + 1) * 128],
                                     rhs=ident[:E + 1, :E + 1],
                                     start=True, stop=True)
                    if c % 2 == 0:
                        nc.vector.tensor_copy(out=Ut_all[:, jt, :], in_=tp)
                    else:
                        nc.scalar.copy(out=Ut_all[:, jt, :], in_=tp)

        # ---- Phase D: final reduction to (masked_sum, count) ----
        with tc.tile_pool(name="fin", bufs=1) as fin, \
             tc.tile_pool(name="fps", bufs=1, space="PSUM") as fps:
            scr = fin.tile([128, BT, E], F32)
            w2 = fin.tile([128, 1], F32)
            nc.vector.tensor_tensor_reduce(
                out=scr, in0=P_all, in1=Ut_all[:, :, 0:E], scale=1.0,
                scalar=0.0, op0=OP.mult, op1=OP.add, accum_out=w2)
            scr2 = fin.tile([128, BT], F32)
            nedot = fin.tile([128, 1], F32)
            nc.vector.tensor_tensor_reduce(
                out=scr2, in0=ne_all, in1=Ut_all[:, :, E], scale=1.0,
                scalar=0.0, op0=OP.mult, op1=OP.add, accum_out=nedot)
            accs = fin.tile([128, 2], F32)
            nc.vector.tensor_tensor(accs[:, 0:1], nedot, w2, op=OP.subtract)
            nc.vector.reduce_sum(out=accs[:, 1:2], in_=Ut_all[:, :, E], axis=AX)
            res = fps.tile([1, 2], F32)
            nc.tensor.matmul(out=res, lhsT=ones, rhs=accs, start=True,
                             stop=True)
            out_sb = fin.tile([1, 2], F32)
            nc.scalar.copy(out=out_sb, in_=res)
            nc.sync.dma_start(out=out_dram, in_=out_sb)


def build_bass():
    nc = bass.Bass("TRN2", target_bir_lowering=False, debug=False)
    emb = nc.dram_tensor("emb", [B, H], F32, kind="ExternalInput").ap()
    emb_s = nc.dram_tensor("emb_strip", [STRIP, H], F32,
                           kind="ExternalInput").ap()
    rp = nc.dram_tensor("rp", [B, E], F32, kind="ExternalInput").ap()
    rp_s = nc.dram_tensor("rp_strip", [STRIP, E], F32,
                          kind="ExternalInput").ap()
    out = nc.dram_tensor("out", [1, 2], F32, kind="ExternalOutput").ap()
    with tile.TileContext(nc) as tc:
        _kernel(tc, emb, emb_s, rp, rp_s, out)
    return nc


_NC_CACHE = None


def kernel(routing_probs: np.ndarray, input_embeddings: np.ndarray,
           **_unused) -> np.ndarray:
    global _NC_CACHE
    if _NC_CACHE is None:
        _NC_CACHE = build_bass()
    nc = _NC_CACHE
    rp = np.ascontiguousarray(routing_probs, dtype=np.float32)
    emb = np.ascontiguousarray(input_embeddings, dtype=np.float32)
    in_maps = []
    for d in range(NCORES):
        in_maps.append({
            "emb": emb,
            "emb_strip": np.ascontiguousarray(emb[d * STRIP:(d + 1) * STRIP]),
            "rp": rp,
            "rp_strip": np.ascontiguousarray(rp[d * STRIP:(d + 1) * STRIP]),
        })
    res = run_bass_kernel_spmd(nc, in_maps, core_ids=list(range(NCORES)))
    vals = np.array([r["out"].reshape(2) for r in res.results],
                    dtype=np.float64)
    total = vals[:, 0].sum()
    cnt = vals[:, 1].sum() - B  # drop the diagonal pairs (kl there is 0)
    if cnt > 0:
        loss = np.float32(total) / np.float32(max(cnt, 1.0))
    else:
        loss = 0.0
    return np.array(WEIGHT * loss, dtype=np.float32)


# revision 13
# speedup vs baseline: 5.8945x; 5.8945x over previous
"""Trainium2 Bass kernel for the batch ConsistencyLoss (masked pairwise KL).

Math (reference):
    emb = x / ||x||;  sim = emb @ emb.T;  mask = (sim > 0.8) & ~eye
    L = log_softmax(routing);  P = exp(L);  ne[j] = sum_k P[j,k] L[j,k]
    kl[i,j] = ne[j] - (L @ P.T)[i,j]
    loss = sum(mask * kl) / count(mask)

Factorization used on device (per core, row strip S of 1024 rows):
    sum_{i in S, j} mask[i,j] * kl[i,j]
      = sum_j ne[j] * colcount_S[j] - sum_{j,k} P[j,k] * (mask_S^T @ L_S)[j,k]
    With Lpad = [L_S | 1] (bf16, [i,17]) one PSUM-accumulated matmul
    U = Lpad^T @ mask_tile gives both terms: U[k,j] (k<16) and colcount in
    U[16,j].  Diagonal pairs have kl == 0 exactly, so they are left in the
    mask and only the count is corrected by -B on the host.

Per core: big bf16 matmul sim_strip = emb_S @ emb^T ([1024, 8192], K=1024),
thresholded straight out of PSUM into a bf16 0/1 mask, tiny U matmul
epilogue.  Embedding normalize+transpose is fused into PE transpose-matmuls
against diag(1/||x||).  Host sums the 8 (sum, count) pairs.
"""

import numpy as np

import concourse.bacc as bacc
import concourse.tile as tile
from concourse import mybir
from concourse.bass_utils import run_bass_kernel_spmd
from concourse.masks import make_identity

B, E, H = 8192, 16, 1024
NCORES = 8
STRIP = B // NCORES  # 1024 rows per core
MT = STRIP // 128    # 8 row chunks per strip
KT = H // 128        # 8 contraction tiles
NT = B // 512        # 16 column tiles of 512
BT = B // 128        # 64 batch tiles
SIM_THRESHOLD = 0.8
WEIGHT = 1.0
F32 = mybir.dt.float32
BF16 = mybir.dt.bfloat16
AX = mybir.AxisListType.X
AXY = mybir.AxisListType.XY
OP = mybir.AluOpType
AF = mybir.ActivationFunctionType


def _softmax_stats(nc, pool, x, negP_out=None, ne_out=None, L_out=None):
    """From logits tile x [128, E]: optionally write -P (f32), ne (f32
    [128,1], ne = sum_k P log P) and L (any dtype) tiles."""
    negmax = pool.tile([128, 1], F32, tag="negmax")
    nc.vector.reduce_max(out=negmax, in_=x, axis=AX, negate=True)
    e = pool.tile([128, E], F32, tag="e")
    s = pool.tile([128, 1], F32, tag="s")
    nc.scalar.activation(out=e, in_=x, func=AF.Exp, bias=negmax, scale=1.0,
                         accum_out=s)
    logs = pool.tile([128, 1], F32, tag="logs")
    nc.scalar.activation(out=logs, in_=s, func=AF.Ln)
    if L_out is not None:
        # L = (x + negmax) - log(sum)
        nc.vector.tensor_scalar(L_out, x, negmax, logs, op0=OP.add,
                                op1=OP.subtract)
    if negP_out is not None:
        rs = pool.tile([128, 1], F32, tag="rs")
        nc.vector.reciprocal(out=rs, in_=s)
        nc.vector.tensor_scalar(negP_out, e, rs, -1.0, op0=OP.mult,
                                op1=OP.mult)
    if ne_out is not None:
        L = pool.tile([128, E], F32, tag="Lf")
        nc.vector.tensor_scalar(L, x, negmax, logs, op0=OP.add, op1=OP.subtract)
        scr = pool.tile([128, E], F32, tag="nescr")
        nc.vector.tensor_tensor(out=scr, in0=negP_out, in1=L, op=OP.mult)
        # scr = -P*L; negate the reduction to get ne = +sum P*L
        nc.vector.reduce_sum(out=ne_out, in_=scr, axis=AX, negate=True)


def _kernel(tc, emb, emb_s, rp, rp_s, out_dram, reps=1):
    nc = tc.nc
    with tc.tile_pool(name="persist", bufs=1) as persist:
        embt = persist.tile([128, KT, B], BF16)        # [h%128, h//128, b]
        stript = persist.tile([128, KT, STRIP], BF16)  # strip columns
        # W17[:, bt, 0:E] = -P, W17[:, bt, E] = ne — matches Ut_all layout so
        # the final masked-sum is one elementwise mult + one reduction.
        W17 = persist.tile([128, BT, E + 1], F32)
        Lpad = persist.tile([128, MT, E + 1], BF16)
        Ut_all = persist.tile([128, BT, E + 1], F32)
        ident = persist.tile([128, 128], F32)
        ones = persist.tile([128, 1], F32)
        make_identity(nc, ident)
        nc.vector.memset(ones, 1.0)
        nc.vector.memset(Lpad[:, :, E], 1.0)

        for rep in range(reps):
            r = f"r{rep}_" if reps > 1 else ""

            # ---- Phase A: softmax stats (full batch -P/ne; strip Lpad) ----
            with tc.tile_pool(name=f"{r}smx", bufs=3) as smx:
                for bt in range(BT):
                    x = smx.tile([128, E], F32, tag="rpx")
                    nc.sync.dma_start(out=x,
                                      in_=rp[bt * 128:(bt + 1) * 128, :])
                    _softmax_stats(nc, smx, x, negP_out=W17[:, bt, 0:E],
                                   ne_out=W17[:, bt, E:E + 1])
                for ms in range(MT):
                    x = smx.tile([128, E], F32, tag="rpx")
                    nc.sync.dma_start(out=x,
                                      in_=rp_s[ms * 128:(ms + 1) * 128, :])
                    _softmax_stats(nc, smx, x, L_out=Lpad[:, ms, 0:E])

            # ---- Phase B: normalize + transpose embeddings ----
            with tc.tile_pool(name=f"{r}embp", bufs=3) as ep, \
                 tc.tile_pool(name=f"{r}trps", bufs=4, space="PSUM") as trps:

                def prep(src_ap, dst_tile, nb):
                    for bt in range(nb):
                        x = ep.tile([128, H], F32, tag="ex")
                        nc.sync.dma_start(
                            out=x, in_=src_ap[bt * 128:(bt + 1) * 128, :])
                        sq = ep.tile([128, H], F32, tag="sq")
                        ss = ep.tile([128, 1], F32, tag="ss")
                        nc.scalar.activation(out=sq, in_=x, func=AF.Square,
                                             bias=0.0, scale=1.0,
                                             accum_out=ss)
                        norm = ep.tile([128, 1], F32, tag="norm")
                        nc.scalar.sqrt(out=norm, in_=ss)
                        rn = ep.tile([128, 1], F32, tag="rn")
                        nc.vector.reciprocal(out=rn, in_=norm)
                        # diag(1/||x||) for this 128-row chunk
                        ds = ep.tile([128, 128], F32, tag="ds")
                        nc.vector.tensor_scalar(ds, ident, rn, None,
                                                op0=OP.mult)
                        for kt in range(KT):
                            tp = trps.tile([128, 128], F32, tag="tr")
                            # out[h, b] = x[b, h] / ||x[b]||
                            nc.tensor.matmul(
                                out=tp, lhsT=x[:, kt * 128:(kt + 1) * 128],
                                rhs=ds, start=True, stop=True)
                            dst = dst_tile[:, kt, bt * 128:(bt + 1) * 128]
                            if kt % 2 == 0:
                                nc.vector.tensor_copy(out=dst, in_=tp)
                            else:
                                nc.scalar.copy(out=dst, in_=tp)

                prep(emb, embt, BT)
                prep(emb_s, stript, MT)

            # ---- Phase C: sim strip matmul + mask + U accumulation ----
            with tc.tile_pool(name=f"{r}simps", bufs=3, space="PSUM") as sps, \
                 tc.tile_pool(name=f"{r}ups", bufs=2, space="PSUM") as ups, \
                 tc.tile_pool(name=f"{r}utps", bufs=2, space="PSUM") as utps, \
                 tc.tile_pool(name=f"{r}mkp", bufs=3) as mkp, \
                 tc.tile_pool(name=f"{r}stg", bufs=2) as stg:
                for n in range(NT):
                    u = ups.tile([E + 1, 512], F32, tag="u")
                    for m in range(MT):
                        sim = sps.tile([128, 512], F32, tag="sim")
                        for kt in range(KT):
                            nc.tensor.matmul(
                                out=sim,
                                lhsT=stript[:, kt, m * 128:(m + 1) * 128],
                                rhs=embt[:, kt, n * 512:(n + 1) * 512],
                                start=(kt == 0), stop=(kt == KT - 1))
                        msk = mkp.tile([128, 512], BF16, tag="mask")
                        nc.vector.tensor_scalar(msk, sim, SIM_THRESHOLD, None,
                                                op0=OP.is_gt)
                        nc.tensor.matmul(out=u, lhsT=Lpad[:, m, :], rhs=msk,
                                         start=(m == 0), stop=(m == MT - 1))
                    # stage U and transpose 128-column blocks into Ut_all
                    ust = stg.tile([E + 1, 512], F32, tag="ust")
                    nc.scalar.copy(out=ust, in_=u)
                    for c in range(4):
                        jt = n * 4 + c
                        tp = utps.tile([128, E + 1], F32, tag="ut")
                        nc.tensor.matmul(out=tp,
                                         lhsT=ust[:, c * 128:(c + 1) * 128],
                                         rhs=ident[:E + 1, :E + 1],
                                         start=True, stop=True)
                        if c % 2 == 0:
                            nc.vector.tensor_copy(out=Ut_all[:, jt, :], in_=tp)
                        else:
                            nc.scalar.copy(out=Ut_all[:, jt, :], in_=tp)

            # ---- Phase D: final reduction to (masked_sum, count) ----
            with tc.tile_pool(name=f"{r}fin", bufs=1) as fin, \
                 tc.tile_pool(name=f"{r}fps", bufs=1, space="PSUM") as fps:
                scr = fin.tile([128, BT, E + 1], F32)
                nc.vector.tensor_tensor(out=scr, in0=W17, in1=Ut_all,
                                        op=OP.mult)
                accs = fin.tile([128, 2], F32)
                nc.vector.reduce_sum(out=accs[:, 0:1], in_=scr, axis=AXY)
                nc.vector.reduce_sum(out=accs[:, 1:2],
                                     in_=Ut_all[:, :, E:E + 1], axis=AXY)
                res = fps.tile([1, 2], F32)
                nc.tensor.matmul(out=res, lhsT=ones, rhs=accs, start=True,
                                 stop=True)
                out_sb = fin.tile([1, 2], F32)
                nc.scalar.copy(out=out_sb, in_=res)
                nc.sync.dma_start(out=out_dram, in_=out_sb)


def build_bass(reps=1):
    nc = bacc.Bacc("TRN2", target_bir_lowering=False, debug=False)
    emb = nc.dram_tensor("emb", [B, H], F32, kind="ExternalInput").ap()
    emb_s = nc.dram_tensor("emb_strip", [STRIP, H], F32,
                           kind="ExternalInput").ap()
    rp = nc.dram_tensor("rp", [B, E], F32, kind="ExternalInput").ap()
    rp_s = nc.dram_tensor("rp_strip", [STRIP, E], F32,
                          kind="ExternalInput").ap()
    out = nc.dram_tensor("out", [1, 2], F32, kind="ExternalOutput").ap()
    with tile.TileContext(nc) as tc:
        _kernel(tc, emb, emb_s, rp, rp_s, out, reps=reps)
    nc.compile()
    return nc


_NC_CACHE = None


def kernel(routing_probs: np.ndarray, input_embeddings: np.ndarray,
           **_unused) -> np.ndarray:
    global _NC_CACHE
    if _NC_CACHE is None:
        _NC_CACHE = build_bass()
    nc = _NC_CACHE
    rp = np.ascontiguousarray(routing_probs, dtype=np.float32)
    emb = np.ascontiguousarray(input_embeddings, dtype=np.float32)
    in_maps = []
    for d in range(NCORES):
        in_maps.append({
            "emb": emb,
            "emb_strip": np.ascontiguousarray(emb[d * STRIP:(d + 1) * STRIP]),
            "rp": rp,
            "rp_strip": np.ascontiguousarray(rp[d * STRIP:(d + 1) * STRIP]),
        })
    res = run_bass_kernel_spmd(nc, in_maps, core_ids=list(range(NCORES)))
    vals = np.array([r["out"].reshape(2) for r in res.results],
                    dtype=np.float64)
    total = vals[:, 0].sum()
    cnt = vals[:, 1].sum() - B  # drop the diagonal pairs (kl there is 0)
    if cnt > 0:
        loss = np.float32(total) / np.float32(max(cnt, 1.0))
    else:
        loss = 0.0
    return np.array(WEIGHT * loss, dtype=np.float32)
